# revision 1
# baseline (speedup 1.0000x reference)
"""Multi-head attention (B=2, L=2048, D=1024, H=16) on 8 Trainium2 NeuronCores.

Sharding: tensor-parallel over heads. Core c owns heads 2c, 2c+1, i.e. rows
[128c, 128c+128) of Wq/Wk/Wv and columns [128c, 128c+128) of Wo. Each core
computes Q/K/V projections for its 128 channels over all 4096 tokens,
attention for its 2 heads (both batches), and a partial out-projection
y_c = attnO_c @ Wo[:, sl].T. The host sums the 8 partials and adds bo
(the gather/unshard step).

Device-side layout notes:
- Activations arrive pre-transposed (host): qT/kT/vT are [D, B*L] so the
  contraction dim d lands on SBUF partitions without any on-device transpose.
- Scores are computed transposed (S.T tiles [k,q]) so softmax P.T lands in
  SBUF ready to be the PV matmul's moving operand; softmax-over-partitions is
  avoided by skipping the max-subtraction (scores are ~N(0,1); exp cannot
  overflow fp32) and computing row sums r with ones-matmuls on the PE.
- The key-padding mask folds into the exp: bias is 0 / -30000 per k-token,
  added per-partition by the ACT instruction, so masked keys exp to 0.0.
- PV packs the two heads into one PSUM bank via column tiling; since
  start=True clears has_written for the whole bank, shared banks are
  pre-cleared with a zero dummy matmul and all real matmuls accumulate.
- Normalization 1/r is broadcast across partitions with a small K=33 matmul
  (compute engines cannot move data across partitions).
"""

import os
import sys

for _p in ("/opt/trn_rl_repo", os.path.expanduser("~/.axon_site/_ro/trn_rl_repo")):
    if os.path.isdir(_p) and _p not in sys.path:
        sys.path.insert(0, _p)

import numpy as np

B = 2
L = 2048
D = 1024
T = B * L            # 4096 tokens
E = 128              # channels per core (2 heads x 64)
HD = 64              # head dim
N_CORES = 8
SCALE = 1.0 / 8.0    # 1/sqrt(HD)
MASK_BIAS = -30000.0

N_TT = T // 512      # 8 projection token tiles
N_DC = D // 128      # 8 contraction chunks
N_QT = L // 512      # 4 query tiles per batch
N_KT = L // 128      # 16 key tiles per batch

_cached = {}


def _build_program(has_bq, has_bk, has_bv, reps=1, mm_dt="f32r", in_dt="bf16"):
    import concourse.bacc as bacc
    import concourse.mybir as mybir
    import concourse.tile as tile

    F32 = mybir.dt.float32
    AF = mybir.ActivationFunctionType
    ALU = mybir.AluOpType
    R = mybir.dt.float32r if mm_dt == "f32r" else mybir.dt.float32
    IDT = mybir.dt.bfloat16 if in_dt == "bf16" else F32

    def mm(out, lhsT, rhs, **kw):
        nc.tensor.matmul(out, lhsT, rhs, **kw)

    nc = bacc.Bacc("TRN2", target_bir_lowering=False, debug=False, num_devices=N_CORES)
    RD = mybir.dt.float32r if mm_dt == "f32r" else mybir.dt.float32

    qT = nc.dram_tensor("qT", [D, T], IDT, kind="ExternalInput").ap()
    kT = nc.dram_tensor("kT", [D, T], IDT, kind="ExternalInput").ap()
    vT = nc.dram_tensor("vT", [D, T], IDT, kind="ExternalInput").ap()
    wq = nc.dram_tensor("wq", [D, E], IDT, kind="ExternalInput").ap()
    wk = nc.dram_tensor("wk", [D, E], IDT, kind="ExternalInput").ap()
    wv = nc.dram_tensor("wv", [D, E], IDT, kind="ExternalInput").ap()
    wo = nc.dram_tensor("wo", [E, D], RD, kind="ExternalInput").ap()
    mbd = nc.dram_tensor("mb", [128, B * N_KT], F32, kind="ExternalInput").ap()
    onesd = nc.dram_tensor("ones128", [128, 128], F32, kind="ExternalInput").ap()
    identd = nc.dram_tensor("ident", [128, 128], F32, kind="ExternalInput").ap()
    bias_d = {}
    if has_bq:
        bias_d["q"] = nc.dram_tensor("bq", [128, 1], F32, kind="ExternalInput").ap()
    if has_bk:
        bias_d["k"] = nc.dram_tensor("bk", [128, 1], F32, kind="ExternalInput").ap()
    if has_bv:
        bias_d["v"] = nc.dram_tensor("bv", [64, 2], F32, kind="ExternalInput").ap()
    yd = nc.dram_tensor("y", [T, D], F32, kind="ExternalOutput").ap()

    with tile.TileContext(nc) as tc:
        import contextlib
        with contextlib.ExitStack() as ctx:
            const = ctx.enter_context(tc.tile_pool(name="const", bufs=1))
            big = ctx.enter_context(tc.tile_pool(name="big", bufs=1))
            stg = ctx.enter_context(tc.tile_pool(name="stg", bufs=5))
            work = ctx.enter_context(tc.tile_pool(name="work", bufs=3))
            pt_pool = ctx.enter_context(tc.tile_pool(name="ptp", bufs=3))
            psum = ctx.enter_context(tc.tile_pool(name="psum", bufs=2, space="PSUM"))
            psst = ctx.enter_context(tc.tile_pool(name="psst", bufs=2, space="PSUM"))
            psacc = ctx.enter_context(tc.tile_pool(name="psacc", bufs=2, space="PSUM"))

            # ---- constants / weights ----
            w_sb = {}
            for wi, (nm, src) in enumerate((("q", wq), ("k", wk), ("v", wv))):
                w = const.tile([128, D], IDT, name=f"w{nm}_sb")
                eng = nc.sync if wi % 2 else nc.scalar
                eng.dma_start(w.rearrange("p (c e) -> p c e", c=N_DC),
                              src[:].rearrange("(c p) e -> p c e", p=128))
                w_sb[nm] = w
            wo_sbA = const.tile([64, D], R, name="wo_sbA")
            nc.sync.dma_start(wo_sbA[:], wo[0:64, :])
            wo_sbB = const.tile([64, D], R, name="wo_sbB")
            nc.scalar.dma_start(wo_sbB[:], wo[64:128, :])
            mb_sb = const.tile([128, B * N_KT], F32, name="mb_sb")
            nc.sync.dma_start(mb_sb[:], mbd[:])
            ones_sb = const.tile([128, 128], F32, name="ones_sb")
            nc.sync.dma_start(ones_sb[:], onesd[:])
            ident_sb = const.tile([128, 128], F32, name="ident_sb")
            nc.sync.dma_start(ident_sb[:], identd[:])
            b_sb = {}
            for nm, src in bias_d.items():
                bt = const.tile([128, 2] if nm == "v" else [128, 1], F32,
                                name=f"b{nm}_sb")
                nc.sync.dma_start(bt[:], src[:])
                b_sb[nm] = bt

            # ---- persistent activations (feature-major) ----
            QT = big.tile([128, T], R, name="QT")   # [e, tok]
            KT = big.tile([128, T], R, name="KT")   # [e, tok]
            # V layout per 128-token group g: [V_headA(64) | 1 | V_headB(64) | 1]
            # the ones column rides along in the PV matmul and produces the
            # softmax denominator as psum row 64.
            V = big.tile([128, 32 * 130], R, name="V")
            for g in range(32):
                nc.gpsimd.memset(V.bitcast(F32)[:, g * 130 + 64:g * 130 + 65], 1.0)
                nc.gpsimd.memset(V.bitcast(F32)[:, g * 130 + 129:g * 130 + 130], 1.0)
            OTA = big.tile([64, T], R, name="OTA")  # [e0:64, tok] normalized
            OTB = big.tile([64, T], R, name="OTB")  # [e64:128, tok] normalized
            rr = big.tile([128, 1024], F32, name="rr")  # 1/r at partition 64

            for rep in range(reps):
                srcs = {"q": qT, "k": kT, "v": vT}

                def proj_unit(nm, tt):
                    # one strided DMA lands all 8 contraction chunks
                    # side-by-side: s[p, dc*512 + t] = src[dc*128 + p, tt*512 + t]
                    s = stg.tile([128, 4096], IDT,
                                 name=f"{rep}_stg_{nm}_{tt}", tag="stg")
                    eng = nc.sync if nm != "k" else nc.scalar
                    eng.dma_start(
                        s.rearrange("p (c t) -> p c t", c=N_DC),
                        srcs[nm][:, tt * 512:(tt + 1) * 512]
                        .rearrange("(c p) t -> p c t", p=128))
                    ps = psum.tile([128, 512], F32,
                                   name=f"{rep}_ps_{nm}_{tt}", tag="mm")
                    for dc in range(N_DC):
                        mm(ps[:], w_sb[nm][:, dc * 128:(dc + 1) * 128],
                           s[:, dc * 512:(dc + 1) * 512],
                           start=(dc == 0), stop=(dc == N_DC - 1))
                    if nm in ("q", "k"):
                        dst = (QT if nm == "q" else KT)[:, tt * 512:(tt + 1) * 512]
                        if nm in b_sb:
                            nc.vector.tensor_scalar(dst, ps[:], b_sb[nm][:, 0:1],
                                                    None, ALU.add)
                        else:
                            nc.vector.tensor_copy(dst, ps[:])
                    else:
                        vs = work.tile([128, 512], F32, name=f"{rep}_vs_{tt}",
                                       tag="vs")
                        nc.vector.tensor_copy(vs[:], ps[:])
                        for si in range(4):
                            vtp = psum.tile([128, 128], F32,
                                            name=f"{rep}_vtp_{tt}_{si}", tag="mm")
                            nc.tensor.transpose(vtp[:],
                                                vs[:, si * 128:(si + 1) * 128],
                                                ident_sb[:])
                            g = tt * 4 + si
                            nc.vector.tensor_copy(
                                V[:, g * 130:g * 130 + 64], vtp[:, 0:64])
                            nc.vector.tensor_copy(
                                V[:, g * 130 + 65:g * 130 + 129], vtp[:, 64:128])

                # Software-pipelined attention for one (batch, 512-query tile):
                # scores for k-tile kt+1 are issued on the PE before the PV/r
                # accumulation of k-tile kt, so the PE never stalls on the ACT
                # exp of the tile it just produced.
                def attn_steps(b, qt):
                    q0 = b * L + qt * 512
                    ot = [psacc.tile([65, 512], F32,
                                     name=f"{rep}_ot{h}_{b}_{qt}", tag="acc")
                          for h in range(2)]

                    def scores(kt):
                        k0 = b * L + kt * 128
                        g = b * N_KT + kt
                        s = psst.tile([128, 1024], F32,
                                      name=f"{rep}_st_{b}_{qt}_{kt}", tag="st")
                        for h in range(2):
                            mm(s[:, h * 512:(h + 1) * 512],
                               KT[h * 64:(h + 1) * 64, k0:k0 + 128],
                               QT[h * 64:(h + 1) * 64, q0:q0 + 512],
                               start=True, stop=True)
                        p = pt_pool.tile([128, 1024], R,
                                         name=f"{rep}_pt_{b}_{qt}_{kt}", tag="pt")
                        nc.scalar.activation(p[:], s[:], AF.Exp,
                                             bias=mb_sb[:, g:g + 1],
                                             scale=SCALE)
                        return p

                    def pv(kt, p, last):
                        g = b * N_KT + kt
                        for h in range(2):
                            mm(ot[h][:],
                               V[:, g * 130 + 65 * h: g * 130 + 65 * (h + 1)],
                               p[:, h * 512:(h + 1) * 512],
                               start=(kt == 0), stop=last)

                    pprev = scores(0)
                    for kt in range(1, N_KT):
                        if kt % 4 == 0:
                            yield
                        pcur = scores(kt)
                        pv(kt - 1, pprev, last=False)
                        pprev = pcur
                    pv(N_KT - 1, pprev, last=True)

                    # normalize: OT_h = ot_h[0:64] * broadcast(1 / ot_h[64])
                    with nc.allow_low_precision(reason="feeds f32r matmul"):
                        nc.vector.reciprocal(rr[64:65, 0:512], ot[0][64:65, :])
                        nc.vector.reciprocal(rr[64:65, 512:1024], ot[1][64:65, :])
                    for h, OTh in ((0, OTA), (1, OTB)):
                        bc_ps = psum.tile([64, 512], F32,
                                          name=f"{rep}_bc{h}_{b}_{qt}", tag="mm")
                        mm(bc_ps[:], ones_sb[64:65, 0:64],
                           rr[64:65, h * 512:(h + 1) * 512],
                           start=True, stop=True)
                        bc_sb = work.tile([64, 512], F32,
                                          name=f"{rep}_bcs{h}_{b}_{qt}",
                                          tag=f"bcs{h}")
                        nc.vector.tensor_copy(bc_sb[:], bc_ps[:])
                        dst = OTh[:, q0:q0 + 512]
                        nc.vector.tensor_mul(dst, ot[h][0:64, :], bc_sb[:])
                        if "v" in b_sb:
                            nc.vector.tensor_scalar(dst, dst,
                                                    b_sb["v"][0:64, h:h + 1],
                                                    None, ALU.add)

                def attn(b, qt):
                    for _ in attn_steps(b, qt):
                        pass

                def y_unit(g):
                    yp = psst.tile([128, 1024], F32, name=f"{rep}_yp_{g}", tag="st")
                    for nn in range(2):
                        mm(yp[:, nn * 512:(nn + 1) * 512],
                           OTA[:, g * 128:(g + 1) * 128],
                           wo_sbA[:, nn * 512:(nn + 1) * 512],
                           start=True, stop=False)
                        mm(yp[:, nn * 512:(nn + 1) * 512],
                           OTB[:, g * 128:(g + 1) * 128],
                           wo_sbB[:, nn * 512:(nn + 1) * 512],
                           start=False, stop=True)
                    ys = work.tile([128, 1024], F32, name=f"{rep}_ys_{g}", tag="ys")
                    # during attention ACT is exp-bound: keep copies on DVE;
                    # in the tail both engines are free, so alternate.
                    if g < 22 or g % 2:
                        nc.vector.tensor_copy(ys[:], yp[:])
                    else:
                        nc.scalar.copy(ys[:], yp[:])
                    deng = nc.sync if g % 2 else nc.scalar
                    deng.dma_start(yd[g * 128:(g + 1) * 128, :], ys[:])

                # Interleaved emission: every attention tile is cut into
                # four 4-ktile chunks (generator yields) and exactly one
                # DMA/projection/output unit is emitted per chunk, so the ACT
                # exp stream never starves while the PE does projection work.
                # Emission order is program order: every unit precedes the
                # first chunk that reads its output.
                def units(*specs):
                    for sp in specs:
                        if sp is None:
                            continue
                        kind, a = sp
                        if kind == "p":
                            proj_unit(*a)
                        else:
                            y_unit(a)

                proj_unit("k", 0); proj_unit("q", 0); proj_unit("v", 0)
                P = lambda nm, tt: ("p", (nm, tt))
                Y = lambda g: ("y", g)
                sched = [
                    ((0, 0), [[P("k", 1), P("v", 1)], [P("k", 2), P("v", 2)],
                              [P("k", 3), P("v", 3)], [P("q", 1)]]),
                    ((0, 1), [[P("k", 4)], [P("v", 4)], [P("q", 2)], [P("k", 5)]]),
                    ((0, 2), [[P("v", 5)], [P("q", 3)], [P("k", 6)], [P("v", 6)]]),
                    ((0, 3), [[P("q", 4)], [P("k", 7)], [P("v", 7)], [P("q", 5)]]),
                    ((1, 0), [[P("q", 6), Y(0)], [P("q", 7), Y(1)],
                              [Y(2)], [Y(3)]]),
                    ((1, 1), [[Y(4), Y(5)], [Y(6), Y(7)], [Y(8)], [Y(9)]]),
                    ((1, 2), [[Y(10), Y(11)], [Y(12), Y(13)], [Y(14)], [Y(15)]]),
                    ((1, 3), [[Y(16), Y(17)], [Y(18), Y(19)], [Y(20)], [Y(21)]]),
                ]
                for (b, qt), per in sched:
                    gen = attn_steps(b, qt)
                    for ci in range(4):
                        if ci < 3:
                            next(gen)
                        else:
                            for _ in gen:
                                pass
                        for kind, a in per[ci]:
                            if kind == "p":
                                proj_unit(*a)
                            else:
                                y_unit(a)
                for g in range(22, 32):
                    y_unit(g)

    nc.compile()
    return nc


def _host_prep(q, k, v, mask, Wq, bq, Wk, bk, Wv, bv, Wo, in_dt="bf16"):
    """Build the per-core input maps."""
    import ml_dtypes
    f32 = np.float32
    idt = ml_dtypes.bfloat16 if in_dt == "bf16" else f32
    qT = np.ascontiguousarray(q.reshape(T, D).T.astype(idt))
    kT = np.ascontiguousarray(k.reshape(T, D).T.astype(idt))
    vT = np.ascontiguousarray(v.reshape(T, D).T.astype(idt))
    mb = np.where(mask, f32(MASK_BIAS), f32(0.0)).astype(f32)      # [B, L]
    mb = np.ascontiguousarray(
        np.transpose(mb.reshape(B, N_KT, 128), (2, 0, 1)).reshape(128, B * N_KT))
    ones128 = np.ones((128, 128), f32)
    ident = np.eye(128, dtype=f32)

    in_maps = []
    for c in range(N_CORES):
        sl = slice(c * E, (c + 1) * E)
        m = {
            "qT": qT, "kT": kT, "vT": vT,
            "wq": np.ascontiguousarray(Wq[sl, :].T.astype(idt)),
            "wk": np.ascontiguousarray(Wk[sl, :].T.astype(idt)),
            "wv": np.ascontiguousarray(Wv[sl, :].T.astype(idt)),
            "wo": np.ascontiguousarray(Wo[:, sl].T.astype(f32)),
            "mb": mb, "ones128": ones128, "ident": ident,
        }
        if np.any(bq):
            m["bq"] = np.ascontiguousarray(bq[sl].astype(f32).reshape(128, 1))
        if np.any(bk):
            m["bk"] = np.ascontiguousarray(bk[sl].astype(f32).reshape(128, 1))
        if np.any(bv):
            m["bv"] = np.ascontiguousarray(bv[sl].astype(f32).reshape(2, 64).T)
        in_maps.append(m)
    return in_maps


def _build_floor_program():
    """Near-empty program used to measure the axon dispatch floor."""
    import concourse.bacc as bacc
    import concourse.mybir as mybir
    import concourse.tile as tile
    import contextlib

    F32 = mybir.dt.float32
    nc = bacc.Bacc("TRN2", target_bir_lowering=False, debug=False,
                   num_devices=N_CORES)
    x = nc.dram_tensor("x", [128, 8], F32, kind="ExternalInput").ap()
    y = nc.dram_tensor("yf", [128, 8], F32, kind="ExternalOutput").ap()
    with tile.TileContext(nc) as tc:
        with contextlib.ExitStack() as ctx:
            sb = ctx.enter_context(tc.tile_pool(name="sb", bufs=1))
            t = sb.tile([128, 8], F32, name="t")
            nc.sync.dma_start(t[:], x[:])
            nc.sync.dma_start(y[:], t[:])
    nc.compile()
    return nc


def _make_timed_runner(nc, in_maps):
    """Build a reusable jitted runner for `nc` (no output donation — the
    program writes every output element, so uninit result buffers are fine).
    Returns (run_once() -> per-core outputs as numpy, time_iters(n) -> [sec])."""
    import jax
    import time
    import concourse.mybir as mybir
    from concourse import bass2jax
    from jax.experimental.shard_map import shard_map
    from jax.sharding import Mesh, NamedSharding, PartitionSpec

    bass2jax.install_neuronx_cc_hook()

    partition_name = nc.partition_id_tensor.name if nc.partition_id_tensor else None
    in_names, out_names, out_avals, zero_outs = [], [], [], []
    for alloc in nc.m.functions[0].allocations:
        if not isinstance(alloc, mybir.MemoryLocationSet):
            continue
        name = alloc.memorylocations[0].name
        if alloc.kind == "ExternalInput":
            if name != partition_name:
                in_names.append(name)
        elif alloc.kind == "ExternalOutput":
            shape = tuple(alloc.tensor_shape)
            dtype = mybir.dt.np(alloc.dtype)
            out_names.append(name)
            out_avals.append(jax.core.ShapedArray(shape, dtype))
            zero_outs.append(np.zeros(shape, dtype))
    n_params = len(in_names)
    all_in_names = list(in_names) + list(out_names)
    if partition_name is not None:
        all_in_names.append(partition_name)

    def _body(*args):
        operands = list(args)
        if partition_name is not None:
            operands.append(bass2jax.partition_id_tensor())
        outs = bass2jax._bass_exec_p.bind(
            *operands,
            out_avals=tuple(out_avals),
            in_names=tuple(all_in_names),
            out_names=tuple(out_names),
            lowering_input_output_aliases=(),
            sim_require_finite=True,
            sim_require_nnan=True,
            nc=nc,
        )
        return tuple(outs)

    devices = jax.devices()[:N_CORES]
    mesh = Mesh(np.asarray(devices), ("core",))
    nin = n_params + len(out_names)
    fn = jax.jit(shard_map(_body, mesh=mesh,
                           in_specs=(PartitionSpec("core"),) * nin,
                           out_specs=(PartitionSpec("core"),) * len(out_names),
                           check_rep=False))
    sh = NamedSharding(mesh, PartitionSpec("core"))
    dev_args = [
        jax.device_put(
            np.concatenate([np.asarray(in_maps[c][nm]) for c in range(N_CORES)],
                           axis=0), sh)
        for nm in in_names
    ] + [
        jax.device_put(np.zeros((N_CORES * z.shape[0], *z.shape[1:]), z.dtype), sh)
        for z in zero_outs
    ]

    def run_once():
        outs = fn(*dev_args)
        jax.block_until_ready(outs)
        return [
            {nm: np.asarray(outs[i]).reshape(N_CORES, *out_avals[i].shape)[c]
             for i, nm in enumerate(out_names)}
            for c in range(N_CORES)
        ]

    def time_iters(n):
        ts = []
        for _ in range(n):
            t0 = time.perf_counter()
            jax.block_until_ready(fn(*dev_args))
            ts.append(time.perf_counter() - t0)
        return ts

    _chain_cache = {}

    def _chain_fn(n_chain):
        if n_chain in _chain_cache:
            return _chain_cache[n_chain]

        def _body_chain(*args):
            ins = list(args[:n_params])
            seed = list(args[n_params:])
            for _ in range(n_chain):
                operands = ins + seed
                if partition_name is not None:
                    operands.append(bass2jax.partition_id_tensor())
                seed = list(bass2jax._bass_exec_p.bind(
                    *operands,
                    out_avals=tuple(out_avals),
                    in_names=tuple(all_in_names),
                    out_names=tuple(out_names),
                    lowering_input_output_aliases=(),
                    sim_require_finite=True,
                    sim_require_nnan=True,
                    nc=nc,
                ))
            return tuple(seed)

        f = jax.jit(shard_map(_body_chain, mesh=mesh,
                              in_specs=(PartitionSpec("core"),) * nin,
                              out_specs=(PartitionSpec("core"),) * len(out_names),
                              check_rep=False))
        jax.block_until_ready(f(*dev_args))  # compile + warm
        _chain_cache[n_chain] = f
        return f

    def time_chain(n_chain, reps):
        f = _chain_fn(n_chain)
        ts = []
        for _ in range(reps):
            t0 = time.perf_counter()
            jax.block_until_ready(f(*dev_args))
            ts.append(time.perf_counter() - t0)
        return ts

    return run_once, time_iters, time_chain


def kernel(q, k, v, mask, Wq, bq, Wk, bk, Wv, bv, Wo, bo):
    from concourse.bass_utils import run_bass_kernel_spmd

    q, k, v = (np.asarray(x) for x in (q, k, v))
    mask = np.asarray(mask)
    in_maps = _host_prep(q, k, v, mask, np.asarray(Wq), np.asarray(bq),
                         np.asarray(Wk), np.asarray(bk), np.asarray(Wv),
                         np.asarray(bv), np.asarray(Wo))
    key = (("bq" in in_maps[0]), ("bk" in in_maps[0]), ("bv" in in_maps[0]))
    if key not in _cached:
        _cached[key] = _build_program(*key)
    nc = _cached[key]

    trace = bool(int(os.environ.get("KERNEL_TRACE", "0")))
    res = run_bass_kernel_spmd(nc, in_maps, list(range(N_CORES)), trace=trace)
    kernel.last_results = res

    y = np.zeros((T, D), np.float64)
    for i in range(N_CORES):
        y += res.results[i]["y"].astype(np.float64)
    y = (y + np.asarray(bo).astype(np.float64)).astype(np.float32)
    return y.reshape(B, L, D)



# revision 31
# speedup vs baseline: 1.9199x; 1.9199x over previous
"""Multi-head attention (B=2, L=2048, D=1024, H=16) on 8 Trainium2 NeuronCores.

Sharding: tensor-parallel over heads. Core c owns heads 2c, 2c+1, i.e. rows
[128c, 128c+128) of Wq/Wk/Wv and columns [128c, 128c+128) of Wo. Each core
computes Q/K/V projections for its 128 channels, attention for its 2 heads
(both batches), and a partial transposed out-projection yT_c = (attnO_c @
Wo[:, sl].T).T emitted in bf16. The host sums the 8 partials, transposes,
and adds bo.

Key optimizations over the naive layout:
- Key-padding-mask compaction: masked keys contribute exp(-inf)=0 exactly,
  so the host gathers only unmasked key/value tokens (padded to 128) and the
  kernel sizes its K/V projection + attention loops to the compacted length.
  With the ~50% random mask this halves scores/PV/exp work. Pad slots get
  bias -30000 so they exp to 0.0 like the reference's masked keys.
- All PE operands are bf16 (1 cycle/row; fp32 moving operands cost 4).
  Accumulation stays fp32 in PSUM.
- V is projected directly into [token, channel] layout (contraction chunks
  of x as the stationary operand), avoiding separate PE transposes.
- Scores are computed transposed (S.T tiles [k,q]) so softmax P.T lands in
  SBUF ready to be the PV matmul's moving operand; softmax max-subtraction
  is skipped (scores ~N(0,1), fp32 exp cannot overflow) and row sums ride
  along as a ones-column in the PV stationary, appearing as psum row 64.
- The out-projection is computed transposed (yT [D, T]): stationary
  woT chunks [128e, 128d] give contraction depth 128 (vs 64 the other way),
  halving out-proj PE time, and the bf16 yT output halves output DMA.
- Attention is software-pipelined with lag 2 (scores for kt issued two
  steps ahead of the PV accumulation of kt) so the PE never waits on the
  ACT exp; projection/output units are interleaved at generator yields.
"""

import os
import sys

for _p in ("/opt/trn_rl_repo", os.path.expanduser("~/.axon_site/_ro/trn_rl_repo")):
    if os.path.isdir(_p) and _p not in sys.path:
        sys.path.insert(0, _p)

import numpy as np

B = 2
L = 2048
D = 1024
T = B * L            # 4096 query tokens
E = 128              # channels per core (2 heads x 64)
HD = 64              # head dim
N_CORES = 8
SCALE = 1.0 / 8.0    # 1/sqrt(HD)
MASK_BIAS = -30000.0

N_DC = D // 128      # 8 contraction chunks
N_QT = L // 512      # 4 query tiles per batch

_cached = {}


def _build_program(has_bq, has_bk, has_bv, nkt0, nkt1, reps=1):
    import concourse.bacc as bacc
    import concourse.mybir as mybir
    import concourse.tile as tile

    F32 = mybir.dt.float32
    BF16 = mybir.dt.bfloat16
    AF = mybir.ActivationFunctionType
    ALU = mybir.AluOpType

    n_kt = (nkt0, nkt1)
    NG = nkt0 + nkt1               # 128-token key groups, both batches
    TK = 128 * NG                  # compacted+padded key tokens
    TKP = 512 * ((TK + 511) // 512)  # staged to 512-token proj tiles
    N_KVT = TKP // 512             # k/v projection token tiles
    N_QTT = T // 512               # q projection token tiles (8)
    goff = (0, nkt0)               # group offset per batch

    def mm(out, lhsT, rhs, **kw):
        nc.tensor.matmul(out, lhsT, rhs, **kw)

    nc = bacc.Bacc("TRN2", target_bir_lowering=False, debug=False,
                   num_devices=N_CORES)

    qT = nc.dram_tensor("qT", [D, T], BF16, kind="ExternalInput").ap()
    kT = nc.dram_tensor("kT", [D, TKP], BF16, kind="ExternalInput").ap()
    vT = nc.dram_tensor("vT", [D, TKP], BF16, kind="ExternalInput").ap()
    # w inputs are pre-chunked on the host: w[p, dc*128+e] = W.T[dc*128+p, e]
    # so the load is a plain contiguous DMA (2KB/partition descriptors).
    wq = nc.dram_tensor("wq", [128, D], BF16, kind="ExternalInput").ap()
    wk = nc.dram_tensor("wk", [128, D], BF16, kind="ExternalInput").ap()
    wv = nc.dram_tensor("wv", [128, D], BF16, kind="ExternalInput").ap()
    wo = nc.dram_tensor("wo", [E, D], BF16, kind="ExternalInput").ap()
    mbd = nc.dram_tensor("mb", [128, NG], F32, kind="ExternalInput").ap()
    onesd = nc.dram_tensor("ones128", [128, 128], BF16, kind="ExternalInput").ap()
    bias_d = {}
    if has_bq:
        bias_d["q"] = nc.dram_tensor("bq", [128, 1], F32, kind="ExternalInput").ap()
    if has_bk:
        bias_d["k"] = nc.dram_tensor("bk", [128, 1], F32, kind="ExternalInput").ap()
    if has_bv:
        bias_d["v"] = nc.dram_tensor("bv", [64, 2], F32, kind="ExternalInput").ap()
    yd = nc.dram_tensor("y", [D, T], BF16, kind="ExternalOutput").ap()

    with tile.TileContext(nc) as tc:
        import contextlib
        with contextlib.ExitStack() as ctx:
            const = ctx.enter_context(tc.tile_pool(name="const", bufs=1))
            big = ctx.enter_context(tc.tile_pool(name="big", bufs=1))
            stg = ctx.enter_context(tc.tile_pool(name="stg", bufs=10))
            work = ctx.enter_context(tc.tile_pool(name="work", bufs=6))
            pt_pool = ctx.enter_context(tc.tile_pool(name="ptp", bufs=3))
            psum = ctx.enter_context(tc.tile_pool(name="psum", bufs=2, space="PSUM"))
            psst = ctx.enter_context(tc.tile_pool(name="psst", bufs=2, space="PSUM"))
            psacc = ctx.enter_context(tc.tile_pool(name="psacc", bufs=2, space="PSUM"))

            # ---- weights needed by the first projections (the remaining
            # constants are DMA'd inside rep 0 at scheduled points) ----
            # weight tiles (DMAs are emitted by the rep-0 startup sequence
            # below, in deadline order on a single queue)
            w_sb = {}
            w_src = {"k": wk, "v": wv, "q": wq}
            for nm in ("k", "v", "q"):
                w_sb[nm] = const.tile([128, D], BF16, name=f"w{nm}_sb")
            wo_sb = const.tile([128, D], BF16, name="wo_sb")
            mb_sb = const.tile([128, NG], F32, name="mb_sb")
            ones_sb = const.tile([128, 128], BF16, name="ones_sb")
            b_sb = {}
            for nm in bias_d:
                b_sb[nm] = const.tile([128, 2] if nm == "v" else [128, 1], F32,
                                      name=f"b{nm}_sb")

            # ---- persistent activations ----
            QT = big.tile([128, T], BF16, name="QT")     # [e, q-tok]
            KT = big.tile([128, TKP], BF16, name="KT")   # [e, k-tok]
            # V layout per 128-token group g: [V_headA(64) | 1 | V_headB(64) | 1]
            # partitions = k-token; the ones column rides along in the PV
            # matmul and produces the softmax denominator as psum row 64.
            V = big.tile([128, NG * 130], BF16, name="V")
            for g in range(NG):
                nc.gpsimd.memset(V[:, g * 130 + 64:g * 130 + 65], 1.0)
                nc.gpsimd.memset(V[:, g * 130 + 129:g * 130 + 130], 1.0)
            OT = big.tile([128, T], BF16, name="OT")     # [e(2 heads), q-tok]
            scr = big.tile([1, 8], F32, name="scr")      # ACT table preload out

            for rep in range(reps):
                srcs = {"q": qT, "k": kT, "v": vT}
                staged = {}

                def unit_cols(nm, tt):
                    # trim the zero-padded tail of the compacted k/v stream
                    return 512 if nm == "q" else min(512, TK - tt * 512)

                def proj_dma(nm, tt):
                    # one strided DMA lands all 8 contraction chunks
                    # side-by-side: s[p, dc*cw + t] = src[dc*128 + p, tt*512 + t]
                    # ALL input staging goes on the ONE scalar HWDGE queue:
                    # the DMA device serves transfer requests FIFO and HWDGE
                    # round-robins descriptor-gen between queues, so a
                    # single queue is the only way to control global
                    # transfer order (SWDGE gen is ~1us/DMA on Pool and
                    # loses the FIFO race entirely).
                    cw = unit_cols(nm, tt)
                    s = stg.tile([128, N_DC * cw], BF16,
                                 name=f"{rep}_stg_{nm}_{tt}", tag="stg")
                    nc.scalar.dma_start(
                        s.rearrange("p (c t) -> p c t", c=N_DC),
                        srcs[nm][:, tt * 512:tt * 512 + cw]
                        .rearrange("(c p) t -> p c t", p=128))
                    staged[(nm, tt)] = s

                def proj_mm(nm, tt):
                    s = staged.pop((nm, tt))
                    cw = unit_cols(nm, tt)
                    if nm in ("q", "k"):
                        ps = psum.tile([128, 512], F32,
                                       name=f"{rep}_ps_{nm}_{tt}", tag="mm")
                        for dc in range(N_DC):
                            mm(ps[:, 0:cw],
                               w_sb[nm][:, dc * 128:(dc + 1) * 128],
                               s[:, dc * cw:(dc + 1) * cw],
                               start=(dc == 0), stop=(dc == N_DC - 1))
                        dst = (QT if nm == "q" else KT)[:, tt * 512:tt * 512 + cw]
                        with nc.allow_low_precision(reason="bf16 activations"):
                            if nm in b_sb:
                                nc.vector.tensor_scalar(dst, ps[:, 0:cw],
                                                        b_sb[nm][:, 0:1],
                                                        None, ALU.add)
                            else:
                                nc.vector.tensor_copy(dst, ps[:, 0:cw])
                    else:
                        # V^T directly: out[t, e] accumulated over d-chunks
                        # with the x chunk as the stationary operand.
                        ps = psum.tile([128, 512], F32,
                                       name=f"{rep}_ps_v_{tt}", tag="mm")
                        for si in range(cw // 128):
                            if tt * 4 + si >= NG:
                                break
                            for dc in range(N_DC):
                                mm(ps[:, si * 128:(si + 1) * 128],
                                   s[:, dc * cw + si * 128:dc * cw + (si + 1) * 128],
                                   w_sb["v"][:, dc * 128:(dc + 1) * 128],
                                   start=(dc == 0), stop=(dc == N_DC - 1))
                        with nc.allow_low_precision(reason="bf16 activations"):
                            for si in range(cw // 128):
                                g = tt * 4 + si
                                if g >= NG:
                                    break
                                nc.vector.tensor_copy(
                                    V[:, g * 130:g * 130 + 64],
                                    ps[:, si * 128:si * 128 + 64])
                                nc.vector.tensor_copy(
                                    V[:, g * 130 + 65:g * 130 + 129],
                                    ps[:, si * 128 + 64:si * 128 + 128])

                # Software-pipelined attention for one (batch, 512-query
                # tile): scores for k-tile kt are issued on the PE two steps
                # before the PV accumulation of kt, so the PE never stalls on
                # the ACT exp of the tile it just produced. Yields carry
                # (pe_ns_just_emitted, key_groups_needed_next, at_normalize)
                # so the emitter can account PE time, force the K/V
                # projection units the next scores depend on, and slot
                # fillers into the dependency gaps.
                def attn_steps(b, qt):
                    q0 = b * L + qt * 512
                    nkt = n_kt[b]
                    ot = [psacc.tile([65, 512], F32,
                                     name=f"{rep}_ot{h}_{b}_{qt}", tag="acc")
                          for h in range(2)]

                    def scores(kt):
                        g = goff[b] + kt
                        k0 = g * 128
                        s = psst.tile([128, 1024], F32,
                                      name=f"{rep}_st_{b}_{qt}_{kt}", tag="st")
                        for h in range(2):
                            mm(s[:, h * 512:(h + 1) * 512],
                               KT[h * 64:(h + 1) * 64, k0:k0 + 128],
                               QT[h * 64:(h + 1) * 64, q0:q0 + 512],
                               start=True, stop=True)
                        p = pt_pool.tile([128, 1024], BF16,
                                         name=f"{rep}_pt_{b}_{qt}_{kt}", tag="pt")
                        nc.scalar.activation(p[:], s[:], AF.Exp,
                                             bias=mb_sb[:, g:g + 1],
                                             scale=SCALE)
                        return p

                    def pv(kt, p, last):
                        g = goff[b] + kt
                        for h in range(2):
                            mm(ot[h][:],
                               V[:, g * 130 + 65 * h: g * 130 + 65 * (h + 1)],
                               p[:, h * 512:(h + 1) * 512],
                               start=(kt == 0), stop=last)

                    gg = goff[b]
                    yield (0, [gg, gg + min(1, nkt - 1)], False)
                    pq = [scores(0)]
                    if nkt > 1:
                        pq.append(scores(1))
                    seg = 852
                    for kt in range(2, nkt):
                        if kt % 2 == 0:
                            need = [gg + kt, gg + min(kt + 1, nkt - 1)]
                            yield (seg, need, False)
                            seg = 0
                        pq.append(scores(kt))
                        pv(kt - 2, pq.pop(0), last=False)
                        seg += 852
                    for i, p in enumerate(pq):
                        kt = nkt - len(pq) + i
                        pv(kt, p, last=(kt == nkt - 1))
                        seg += 426

                    # normalize: OT rows = [otA/rA ; otB/rB]; 1/r broadcast
                    # across the 64 head partitions with a K=1 ones matmul
                    # (compute engines cannot move data across partitions).
                    # The yield lets PE fillers run while the DVE computes
                    # the reciprocals, instead of stalling on the bc matmul.
                    rb = [work.tile([65, 512], BF16,
                                    name=f"{rep}_rb{h}_{b}_{qt}", tag=f"rb{h}")
                          for h in range(2)]
                    with nc.allow_low_precision(reason="feeds bf16 matmul"):
                        nc.vector.reciprocal(rb[0][64:65, :], ot[0][64:65, :])
                        nc.vector.reciprocal(rb[1][64:65, :], ot[1][64:65, :])
                    yield (seg, [], True)
                    bc_sb = work.tile([64, 1024], F32,
                                      name=f"{rep}_bcs_{b}_{qt}", tag="bcs")
                    for h in range(2):
                        bc_ps = psum.tile([64, 512], F32,
                                          name=f"{rep}_bc{h}_{b}_{qt}", tag="mm")
                        mm(bc_ps[:], ones_sb[64:65, 0:64], rb[h][64:65, :],
                           start=True, stop=True)
                        nc.vector.tensor_copy(bc_sb[:, h * 512:(h + 1) * 512],
                                              bc_ps[:])
                    for h in range(2):
                        dst = OT[h * 64:(h + 1) * 64, q0:q0 + 512]
                        with nc.allow_low_precision(reason="bf16 attn output"):
                            nc.vector.tensor_mul(
                                dst, ot[h][0:64, :],
                                bc_sb[:, h * 512:(h + 1) * 512])
                        if "v" in b_sb:
                            nc.vector.tensor_scalar(dst, dst,
                                                    b_sb["v"][0:64, h:h + 1],
                                                    None, ALU.add)

                def y_unit(dc, tts, tail=False):
                    # yT tiles [128 d, 512 t]: contraction over all 128
                    # channels in one pass with the woT chunk stationary.
                    # Adjacent token tiles share one output DMA (via the
                    # otherwise-idle Pool engine's SWDGE) to halve the
                    # fixed per-DMA descriptor-generation cost.
                    ys = work.tile([128, 512 * len(tts)], BF16,
                                   name=f"{rep}_ys_{dc}_{tts[0]}", tag="ys")
                    for j, tt in enumerate(tts):
                        # in the tail the scores pool is free: its extra
                        # PSUM banks deepen the mm->copy->DMA rotation
                        yp = (psst if tail else psum).tile(
                            [128, 512], F32, name=f"{rep}_yp_{dc}_{tt}",
                            tag="st" if tail else "mm")
                        mm(yp[:], wo_sb[:, dc * 128:(dc + 1) * 128],
                           OT[:, tt * 512:(tt + 1) * 512],
                           start=True, stop=True)
                        # during attention the ACT engine is exp-bound: keep
                        # copies on the DVE; in the tail both are free.
                        with nc.allow_low_precision(reason="bf16 out"):
                            if tail and (dc + j) % 2 == 0:
                                nc.scalar.copy(ys[:, j * 512:(j + 1) * 512],
                                               yp[:])
                            else:
                                nc.vector.tensor_copy(
                                    ys[:, j * 512:(j + 1) * 512], yp[:])
                    eng = (nc.sync if dc % 2 else nc.scalar) if tail \
                        else nc.gpsimd
                    eng.dma_start(
                        yd[dc * 128:(dc + 1) * 128,
                           tts[0] * 512:(tts[-1] + 1) * 512], ys[:])

                # ---- emission schedule ----
                # DMA transfers are serialized at HBM bandwidth, so the
                # issue order is the data arrival order; a static clock
                # estimate paces issues ~LEAD ns ahead of PE consumption
                # and gates optional filler matmuls on estimated arrival.
                STG_NS, LEAD = 2950.0, 9000.0
                est = {"dma": 0.0, "pe": 0.0}
                ready = {}

                def issue_dma(u):
                    proj_dma(*u)
                    est["dma"] = max(est["dma"], est["pe"]) + STG_NS
                    ready[u] = est["dma"] + 500.0

                def mm_unit(u):
                    est["pe"] = max(est["pe"], ready[u]) + 1707.0
                    proj_mm(*u)

                n_b0 = min(N_KVT, (nkt0 + 3) // 4)
                kv_all = [(nm, tt) for tt in range(N_KVT)
                          for nm in ("k", "v")]
                pre = kv_all[:2 * n_b0]
                rest = kv_all[2 * n_b0:]
                # units in mm-emission order; DMA order (with the consts
                # woven in by deadline) is built separately below.
                dma_order = [("k", 0), ("v", 0), ("q", 0)] + pre[2:]
                dma_order += [("q", 1), ("q", 2), ("q", 3), ("q", 4)][:N_QTT - 1]
                dma_order += rest
                dma_order += [("q", t) for t in range(5, N_QTT)]

                dma_pend = list(dma_order)
                mm_pend = list(dma_order)
                y_pend = []

                def covered_units(groups):
                    need = []
                    for g in groups:
                        tt = (g * 128) // 512
                        for nm in ("k", "v"):
                            u = (nm, tt)
                            if u in mm_pend and u not in need:
                                need.append(u)
                    return need

                # startup DMA sequence, strictly in deadline order on the
                # single scalar queue: each weight/const lands just before
                # its first consumer, each stg tile as early as possible.
                np_pre = len(pre) + 1
                stg_pre, ci = dma_pend[:np_pre], 0

                def const_dma(dst, src):
                    if rep == 0:
                        nc.scalar.dma_start(dst[:], src[:])
                        est["dma"] += 200.0

                const_dma(w_sb["k"], w_src["k"])
                est["dma"] += 550.0
                issue_dma(stg_pre[0])                     # k0
                const_dma(w_sb["v"], w_src["v"])
                est["dma"] += 550.0
                issue_dma(stg_pre[1])                     # v0
                issue_dma(stg_pre[2])                     # q0
                const_dma(mb_sb, mbd)
                for nm, bt in b_sb.items():
                    const_dma(bt, bias_d[nm])
                const_dma(w_sb["q"], w_src["q"])
                est["dma"] += 550.0
                if len(stg_pre) > 3:
                    issue_dma(stg_pre[3])                 # k1
                const_dma(ones_sb, onesd)
                for u in stg_pre[4:]:
                    issue_dma(u)                          # v1, k2, v2, ...
                del dma_pend[:np_pre]
                # load the ACT exp table during the idle startup window
                if rep == 0:
                    nc.scalar.activation(scr[0:1, 0:1], mb_sb[0:1, 0:1],
                                         AF.Exp, scale=1.0)
                for u in [("k", 0), ("v", 0), ("q", 0)]:
                    mm_unit(u)
                    mm_pend.remove(u)
                const_dma(wo_sb, wo)

                tiles = [(b, qt) for b in range(B) for qt in range(N_QT)]
                last_ti = len(tiles) - 1
                for ti in range(len(tiles)):
                    b, qt = tiles[ti]
                    assert ("q", ti) not in mm_pend, f"q{ti} not emitted"
                    for cost, need, at_norm in attn_steps(b, qt):
                        est["pe"] += cost
                        while dma_pend and est["dma"] < est["pe"] + LEAD:
                            issue_dma(dma_pend.pop(0))
                        budget = 1800.0
                        # forced: K/V units the next scores depend on, and
                        # the next tile's q projection by its deadline.
                        forced = covered_units(need)
                        if at_norm and ("q", ti + 1) in mm_pend:
                            forced.append(("q", ti + 1))
                        for u in forced:
                            while u in dma_pend:  # must be issued by now
                                issue_dma(dma_pend.pop(0))
                            mm_unit(u)
                            mm_pend.remove(u)
                            budget -= 1707
                        if at_norm and ti == last_ti:
                            # final normalize: nothing else will fill the
                            # PE while the DVE reciprocals run - drain all
                            # ready y units here (copies on ACT: exp done)
                            for dc, tts in y_pend:
                                y_unit(dc, tts, tail=True)
                            y_pend = []
                            continue
                        # optional fillers: proj mms whose data has landed,
                        # then ready y units (max 2: deeper bursts stall on
                        # the 2-buffer PSUM pool rotation).
                        y_n = 0
                        while budget > 0:
                            pick = None
                            for u in mm_pend[:2]:
                                if (u in ready
                                        and ready[u] <= est["pe"] + 400
                                        and budget >= 1707):
                                    pick = u
                                    break
                            if pick is not None:
                                mm_unit(pick)
                                mm_pend.remove(pick)
                                budget -= 1707
                            elif (y_pend and y_n < 2
                                    and budget >= 250 * len(y_pend[0][1])):
                                dc, tts = y_pend.pop(0)
                                y_unit(dc, tts)
                                cost_y = 250 * len(tts)
                                est["pe"] += cost_y
                                budget -= cost_y
                                y_n += 1
                            else:
                                break
                    # tile ti's tokens now normalized -> y units over its
                    # token tile become ready (pairs finishing at odd ti).
                    if ti % 2 == 1 and ti < 6:
                        y_pend += [(dc, (ti - 1, ti)) for dc in range(N_DC)]
                    elif ti >= 6:
                        y_pend += [(dc, (ti,)) for dc in range(N_DC)]
                for u in list(mm_pend):
                    while u in dma_pend:
                        issue_dma(dma_pend.pop(0))
                    mm_unit(u)
                    mm_pend.remove(u)
                for dc, tts in y_pend:
                    y_unit(dc, tts, tail=True)

    nc.compile()
    return nc


def _host_prep(q, k, v, mask, Wq, bq, Wk, bk, Wv, bv, Wo):
    """Build the per-core input maps. Compacts masked keys out of k/v."""
    import ml_dtypes
    f32 = np.float32
    bf16 = ml_dtypes.bfloat16

    qT = np.ascontiguousarray(q.reshape(T, D).T.astype(bf16))

    # --- key compaction: keep only unmasked tokens, pad groups to 128 ---
    idxs, biases, nkts = [], [], []
    for b in range(B):
        idx = np.flatnonzero(~mask[b])
        nkt = max(1, (len(idx) + 127) // 128)
        pad = 128 * nkt - len(idx)
        bias = np.concatenate([np.zeros(len(idx), f32),
                               np.full(pad, MASK_BIAS, f32)])
        idx = np.concatenate([idx, np.zeros(pad, np.int64)])
        idxs.append(idx)
        biases.append(bias)
        nkts.append(nkt)
    NG = sum(nkts)
    TK = 128 * NG
    TKP = 512 * ((TK + 511) // 512)
    kc = np.concatenate([k[b][idxs[b]] for b in range(B)], axis=0)
    vc = np.concatenate([v[b][idxs[b]] for b in range(B)], axis=0)
    kc = np.concatenate([kc, np.zeros((TKP - TK, D), kc.dtype)], axis=0)
    vc = np.concatenate([vc, np.zeros((TKP - TK, D), vc.dtype)], axis=0)
    kT = np.ascontiguousarray(kc.T.astype(bf16))
    vT = np.ascontiguousarray(vc.T.astype(bf16))
    mb = np.concatenate(biases).reshape(NG, 128).T
    mb = np.ascontiguousarray(mb.astype(f32))
    ones128 = np.ones((128, 128), bf16)

    def chunked(wT):
        # [D, E] -> [128, N_DC*E]: w[p, dc*E + e] = wT[dc*128 + p, e]
        return np.ascontiguousarray(
            wT.reshape(N_DC, 128, E).transpose(1, 0, 2).reshape(128, D))

    in_maps = []
    for c in range(N_CORES):
        sl = slice(c * E, (c + 1) * E)
        m = {
            "qT": qT, "kT": kT, "vT": vT,
            "wq": chunked(Wq[sl, :].T.astype(bf16)),
            "wk": chunked(Wk[sl, :].T.astype(bf16)),
            "wv": chunked(Wv[sl, :].T.astype(bf16)),
            "wo": np.ascontiguousarray(Wo[:, sl].T.astype(bf16)),
            "mb": mb, "ones128": ones128,
        }
        if np.any(bq):
            m["bq"] = np.ascontiguousarray(bq[sl].astype(f32).reshape(128, 1))
        if np.any(bk):
            m["bk"] = np.ascontiguousarray(bk[sl].astype(f32).reshape(128, 1))
        if np.any(bv):
            m["bv"] = np.ascontiguousarray(bv[sl].astype(f32).reshape(2, 64).T)
        in_maps.append(m)
    return in_maps, (nkts[0], nkts[1])


def _make_timed_runner(nc, in_maps):
    """Build a reusable jitted runner for `nc` (no output donation — the
    program writes every output element, so uninit result buffers are fine).
    Returns (run_once() -> per-core outputs as numpy, time_iters(n) -> [sec])."""
    import jax
    import time
    import concourse.mybir as mybir
    from concourse import bass2jax
    from jax.experimental.shard_map import shard_map
    from jax.sharding import Mesh, NamedSharding, PartitionSpec

    bass2jax.install_neuronx_cc_hook()

    partition_name = nc.partition_id_tensor.name if nc.partition_id_tensor else None
    in_names, out_names, out_avals, zero_outs = [], [], [], []
    for alloc in nc.m.functions[0].allocations:
        if not isinstance(alloc, mybir.MemoryLocationSet):
            continue
        name = alloc.memorylocations[0].name
        if alloc.kind == "ExternalInput":
            if name != partition_name:
                in_names.append(name)
        elif alloc.kind == "ExternalOutput":
            shape = tuple(alloc.tensor_shape)
            dtype = mybir.dt.np(alloc.dtype)
            out_names.append(name)
            out_avals.append(jax.core.ShapedArray(shape, dtype))
            zero_outs.append(np.zeros(shape, dtype))
    n_params = len(in_names)
    all_in_names = list(in_names) + list(out_names)
    if partition_name is not None:
        all_in_names.append(partition_name)

    def _body(*args):
        operands = list(args)
        if partition_name is not None:
            operands.append(bass2jax.partition_id_tensor())
        outs = bass2jax._bass_exec_p.bind(
            *operands,
            out_avals=tuple(out_avals),
            in_names=tuple(all_in_names),
            out_names=tuple(out_names),
            lowering_input_output_aliases=(),
            sim_require_finite=True,
            sim_require_nnan=True,
            nc=nc,
        )
        return tuple(outs)

    devices = jax.devices()[:N_CORES]
    mesh = Mesh(np.asarray(devices), ("core",))
    nin = n_params + len(out_names)
    fn = jax.jit(shard_map(_body, mesh=mesh,
                           in_specs=(PartitionSpec("core"),) * nin,
                           out_specs=(PartitionSpec("core"),) * len(out_names),
                           check_rep=False))
    sh = NamedSharding(mesh, PartitionSpec("core"))
    dev_args = [
        jax.device_put(
            np.concatenate([np.asarray(in_maps[c][nm]) for c in range(N_CORES)],
                           axis=0), sh)
        for nm in in_names
    ] + [
        jax.device_put(np.zeros((N_CORES * z.shape[0], *z.shape[1:]), z.dtype), sh)
        for z in zero_outs
    ]

    def run_once():
        outs = fn(*dev_args)
        jax.block_until_ready(outs)
        return [
            {nm: np.asarray(outs[i]).reshape(N_CORES, *out_avals[i].shape)[c]
             for i, nm in enumerate(out_names)}
            for c in range(N_CORES)
        ]

    def time_iters(n):
        ts = []
        for _ in range(n):
            t0 = time.perf_counter()
            jax.block_until_ready(fn(*dev_args))
            ts.append(time.perf_counter() - t0)
        return ts

    return run_once, time_iters


def kernel(q, k, v, mask, Wq, bq, Wk, bk, Wv, bv, Wo, bo):
    from concourse.bass_utils import run_bass_kernel_spmd

    q, k, v = (np.asarray(x) for x in (q, k, v))
    mask = np.asarray(mask)
    in_maps, nkt = _host_prep(q, k, v, mask, np.asarray(Wq), np.asarray(bq),
                              np.asarray(Wk), np.asarray(bk), np.asarray(Wv),
                              np.asarray(bv), np.asarray(Wo))
    key = (("bq" in in_maps[0]), ("bk" in in_maps[0]), ("bv" in in_maps[0]),
           nkt[0], nkt[1])
    if key not in _cached:
        _cached[key] = _build_program(*key)
    nc = _cached[key]

    trace = bool(int(os.environ.get("KERNEL_TRACE", "0")))
    res = run_bass_kernel_spmd(nc, in_maps, list(range(N_CORES)), trace=trace)
    kernel.last_results = res

    yT = np.zeros((D, T), np.float32)
    for i in range(N_CORES):
        yT += res.results[i]["y"].astype(np.float32)
    y = yT.T + np.asarray(bo).astype(np.float32)
    return np.ascontiguousarray(y.astype(np.float32)).reshape(B, L, D)
